# Initial kernel scaffold
#
"""Cross multihead attention (global/local masked head groups) on 8 trn2 cores.

Sharding: core c -> (batch b = c//2, head-group g = c%2).
  g=0: heads 0-7  masked by key_padding_mask[b]
  g=1: heads 8-15 masked by local_mask[b]
Each core computes its group's partial output  (attn_out_g @ Wo[:, g*512:(g+1)*512].T)
of shape [T, E]; the host sums the two partials per batch and adds bo.

On-chip layout ("transposed scores" orientation - zero on-chip transposes):
  qT, kT   : [512(j), 1024(t|s)]  feature-major (j = head*64 + d)
  v        : [1024(s), 8*65]      natural, per-head 65-col stripes [v_h | ones]
  scoresT  : [s, t] tiles; mask folded into Exp bias (per-partition = per-s)
  softmax  : no max-subtraction (scores ~ N(0,1)); denominators from the
             ones column of the augmented v matmul; normalization deferred
             to a per-head [64, 512] multiply with a partition-broadcast
             reciprocal row.
  attnT    : [512(j), 1024(t)] -> out = attnT.T @ woT accumulated over j-tiles.
"""

import os
import sys

sys.path.insert(0, "/opt/trn_rl_repo")

import numpy as np

import concourse.bass as bass
import concourse.mybir as mybir
from concourse.tile import TileContext

B, T, S, E, H = 4, 1024, 1024, 1024, 16
DH = E // H            # 64
HH = H // 2            # 8 heads per group
G = HH * DH            # 512 features per group
SCALING = DH ** -0.5
NEG = -30000.0         # exp(x + NEG) == 0.0 in fp32, no LUT edge cases

F32 = mybir.dt.float32
BF = mybir.dt.bfloat16   # tensor-engine operand dtype (1 cyc/row)


def _mm(ap):
    return ap


def _split_waits(nc):
    """TPB ISA structs hold one sem-wait slot. Tile can emit >1 wait per
    instruction (walrus: 'Too many sync wait commands'); hoist all but the
    last wait onto single-wait NOPs on the same engine, inserted just
    before. Timing is unchanged - the waits would have blocked anyway."""
    k = 0
    for f in nc.m.functions:
        for blk in f.blocks:
            new = []
            for inst in blk.instructions:
                si = inst.sync_info
                w = list(si.on_wait) if si else []
                if len(w) > 1:
                    for wait in w[:-1]:
                        nop = mybir.InstNoOp(name=f"nopw-{k}", ins=[], outs=[])
                        k += 1
                        nop.engine = inst.engine
                        nop.sync_info = mybir.SyncInfo(on_wait=[wait], on_update=[])
                        new.append(nop)
                    inst.sync_info = mybir.SyncInfo(
                        on_wait=[w[-1]], on_update=list(si.on_update)
                    )
                new.append(inst)
            blk.instructions = new
    return nc


def build_nc(split=True, phase='all'):
    nc = bass.Bass()

    xqT = nc.dram_tensor("xqT", [E, T], BF, kind="ExternalInput")
    xkT = nc.dram_tensor("xkT", [E, S], BF, kind="ExternalInput")
    xvT = nc.dram_tensor("xvT", [E, S], BF, kind="ExternalInput")
    wqT = nc.dram_tensor("wqT", [E, G], BF, kind="ExternalInput")
    wkT = nc.dram_tensor("wkT", [E, G], BF, kind="ExternalInput")
    wvT = nc.dram_tensor("wvT", [E, G], BF, kind="ExternalInput")
    woT = nc.dram_tensor("woT", [G, E], BF, kind="ExternalInput")
    mb = nc.dram_tensor("mb", [128, 8], F32, kind="ExternalInput")    # [-30000|0] per s
    bqc = nc.dram_tensor("bqc", [128, 4], F32, kind="ExternalInput")  # bq per j-tile col
    bkc = nc.dram_tensor("bkc", [128, 4], F32, kind="ExternalInput")
    bvr = nc.dram_tensor("bvr", [1, G], BF, kind="ExternalInput")    # bv as row
    out = nc.dram_tensor("out", [T, E], F32, kind="ExternalOutput")

    ET, ST, TT = E // 128, S // 128, T // 128   # 8, 8, 8
    JT = G // 128                               # 4 j-tiles
    NC = 512                                    # moving-operand chunk
    TC = T // NC                                # 2 t-chunks

    with TileContext(nc) as tc:
        with (
            tc.tile_pool(name="const", bufs=1) as pc,
            tc.tile_pool(name="persist", bufs=1) as pp,
            tc.tile_pool(name="xin", bufs=ET) as px,
            tc.tile_pool(name="win", bufs=ET) as pw,
            tc.tile_pool(name="exp", bufs=2 * ST) as pe,
            tc.tile_pool(name="outsb", bufs=3) as po,
            tc.tile_pool(name="small", bufs=4) as psm,
            tc.tile_pool(name="psg", bufs=2, space="PSUM") as ppsg,
            tc.tile_pool(name="pssc", bufs=4, space="PSUM") as ppsc,
            tc.tile_pool(name="psav", bufs=2, space="PSUM") as ppsav,
        ):
            # ---- constants ----
            mb_sb = pc.tile([128, 8], F32, name="mb_sb")
            nc.sync.dma_start(out=mb_sb[:], in_=mb[:])
            bq_sb = pc.tile([128, 4], F32, name="bq_sb")
            nc.sync.dma_start(out=bq_sb[:], in_=bqc[:])
            bk_sb = pc.tile([128, 4], F32, name="bk_sb")
            nc.sync.dma_start(out=bk_sb[:], in_=bkc[:])
            bv_sb = pc.tile([1, G], BF, name="bv_sb")
            nc.sync.dma_start(out=bv_sb[:], in_=bvr[:])
            ones_sb = pc.tile([2, 128], BF, name="ones_sb")
            nc.gpsimd.memset(ones_sb[:], 1.0)

            # ---- persistent activations ----
            qT_sb = [pp.tile([128, T], BF, name=f"qT{r}") for r in range(JT)]
            kT_sb = [pp.tile([128, S], BF, name=f"kT{r}") for r in range(JT)]
            v_sb = [pp.tile([128, HH * (DH + 1)], BF, name=f"v{st}") for st in range(ST)]
            aT_sb = [pp.tile([128, T], BF, name=f"aT{r}") for r in range(JT)]
            woT_sb = [pp.tile([128, E], BF, name=f"woT{r}") for r in range(JT)]

            # ---- q/k projections: out[j,t] = sum_e W.T[e,j] X.T[e,t] (+ bias) ----
            for pi, (xdr, wdr, dst, bias) in enumerate((
                (xqT, wqT, qT_sb, bq_sb),
                (xkT, wkT, kT_sb, bk_sb),
            )):
                xt = [px.tile([128, T], BF, tag=f"xe{pi}", name=f"xe{pi}_{et}") for et in range(ET)]
                wt = [pw.tile([128, G], BF, tag=f"we{pi}", name=f"we{pi}_{et}") for et in range(ET)]
                for et in range(ET):
                    nc.sync.dma_start(out=xt[et][:], in_=xdr[et * 128:(et + 1) * 128, :])
                    nc.sync.dma_start(out=wt[et][:], in_=wdr[et * 128:(et + 1) * 128, :])
                for r in range(JT):
                    for c2 in range(TC):
                        ps = ppsg.tile([128, NC], F32, tag="psg", name="ps_proj")
                        for et in range(ET):
                            nc.tensor.matmul(
                                ps[:],
                                lhsT=_mm(wt[et][:, r * 128:(r + 1) * 128]),
                                rhs=_mm(xt[et][:, c2 * NC:(c2 + 1) * NC]),
                                start=(et == 0), stop=(et == ET - 1),
                            )
                        nc.vector.tensor_scalar_add(
                            dst[r][:, c2 * NC:(c2 + 1) * NC], ps[:], bias[:, r:r + 1]
                        )

            # ---- v projection: v[s,d] = sum_e X.T[e,s] Wv.T[e,d] + bv ----
            xt = [px.tile([128, S], BF, tag="xev", name=f"xve{et}") for et in range(ET)]
            wt = [pw.tile([128, G], BF, tag="wev", name=f"wve{et}") for et in range(ET)]
            for et in range(ET):
                nc.sync.dma_start(out=xt[et][:], in_=xvT[et * 128:(et + 1) * 128, :])
                nc.sync.dma_start(out=wt[et][:], in_=wvT[et * 128:(et + 1) * 128, :])
            for st in range(ST):
                ps = ppsg.tile([128, G], F32, tag="psg", name="ps_v")
                for et in range(ET):
                    nc.tensor.matmul(
                        ps[:],
                        lhsT=_mm(xt[et][:, st * 128:(st + 1) * 128]),
                        rhs=_mm(wt[et][:]),
                        start=(et == 0), stop=False,
                    )
                nc.tensor.matmul(  # += ones[1,128].T @ bv[1,512]
                    ps[:], lhsT=_mm(ones_sb[0:1, :]), rhs=_mm(bv_sb[:]),
                    start=False, stop=True,
                )
                # scatter [128, 8, 64] into 65-col stripes; stripe col 64 <- 1.0
                v3 = v_sb[st][:].rearrange("p (h x) -> p h x", x=DH + 1)
                nc.vector.tensor_copy(
                    v3[:, :, 0:DH], ps[:].rearrange("p (h x) -> p h x", x=DH)
                )
                nc.gpsimd.memset(v3[:, :, DH:DH + 1], 1.0)

            for r in range(JT):
                nc.sync.dma_start(out=woT_sb[r][:], in_=woT[r * 128:(r + 1) * 128, :])

            if phase == 'proj':
                for r in range(JT):
                    ot = po.tile([128, T], F32, tag="otp", name=f"otp{r}")
                    nc.vector.tensor_copy(ot[:], qT_sb[r][:])
                    nc.sync.dma_start(out=out[r * 128:(r + 1) * 128, :], in_=ot[:])
                    ot2 = po.tile([128, T], F32, tag="otp", name=f"otp2{r}")
                    nc.vector.tensor_copy(ot2[:], kT_sb[r][:])
                    nc.sync.dma_start(out=out[512 + r * 128:512 + (r + 1) * 128, :], in_=ot2[:])

            # ---- attention ----
            for c in range(TC if phase == 'all' else 0):
                tsl = slice(c * NC, (c + 1) * NC)
                for hp in range(HH // 2):
                    pair = (2 * hp, 2 * hp + 1)
                    expT = {h: [pe.tile([128, NC], BF, tag="exp", name=f"exp_h{h}_s{st}") for st in range(ST)]
                            for h in pair}
                    for st in range(ST):
                        for h in pair:
                            r, po_ = h // 2, (h % 2) * DH
                            ps_s = ppsc.tile([128, NC], F32, tag="sc", name="ps_s")
                            nc.tensor.matmul(
                                ps_s[:],
                                lhsT=_mm(kT_sb[r][po_:po_ + DH, st * 128:(st + 1) * 128]),
                                rhs=_mm(qT_sb[r][po_:po_ + DH, tsl]),
                                start=True, stop=True,
                            )
                            nc.scalar.activation(
                                expT[h][st][:], ps_s[:],
                                mybir.ActivationFunctionType.Exp,
                                bias=mb_sb[:, st:st + 1], scale=SCALING,
                            )
                    for h in pair:
                        r, po_ = h // 2, (h % 2) * DH
                        ps_o = ppsav.tile([DH + 1, NC], F32, tag="av", name="ps_o")
                        for st in range(ST):
                            nc.tensor.matmul(
                                ps_o[:],
                                lhsT=_mm(v_sb[st][:, h * (DH + 1):(h + 1) * (DH + 1)]),
                                rhs=_mm(expT[h][st][:]),
                                start=(st == 0), stop=(st == ST - 1),
                            )
                        rec = psm.tile([1, NC], F32, tag="rec", name="rec")
                        nc.vector.reciprocal(rec[:], ps_o[DH:DH + 1, :])
                        # broadcast rec across 64 partitions at ~fp32 precision:
                        # hi = bf16(rec), lo = bf16(rec - hi);  ones[2,64].T @ [hi;lo]
                        # sums hi+lo in fp32 PSUM.
                        rhi = psm.tile([1, NC], BF, tag="rhi", name="rhi")
                        nc.vector.tensor_copy(rhi[:], rec[:])
                        rlo = psm.tile([1, NC], BF, tag="rlo", name="rlo")
                        nc.vector.tensor_sub(rlo[:], rec[:], rhi[:])
                        ps_b = ppsc.tile([DH, NC], F32, tag="sc", name="ps_b")
                        nc.tensor.matmul(ps_b[:], lhsT=ones_sb[0:1, 0:DH],
                                         rhs=rhi[:], start=True, stop=False)
                        nc.tensor.matmul(ps_b[:], lhsT=ones_sb[0:1, 0:DH],
                                         rhs=rlo[:], start=False, stop=True)
                        rb = psm.tile([DH, NC], F32, tag="rb", name="rb")
                        nc.vector.tensor_copy(rb[:], ps_b[:])
                        nc.vector.tensor_mul(
                            aT_sb[r][po_:po_ + DH, tsl],
                            ps_o[0:DH, :],
                            rb[:],
                        )
                # ---- output projection for this chunk's t-tiles ----
                for tt in range(c * 4, c * 4 + 4):
                    for oc in range(2):
                        ps_u = ppsg.tile([128, NC], F32, tag="psg", name="ps_u")
                        for r in range(JT):
                            nc.tensor.matmul(
                                ps_u[:],
                                lhsT=_mm(aT_sb[r][:, tt * 128:(tt + 1) * 128]),
                                rhs=_mm(woT_sb[r][:, oc * NC:(oc + 1) * NC]),
                                start=(r == 0), stop=(r == JT - 1),
                            )
                        ot = po.tile([128, NC], F32, tag="ot", name="ot")
                        nc.vector.tensor_copy(ot[:], ps_u[:])
                        nc.sync.dma_start(
                            out=out[tt * 128:(tt + 1) * 128, oc * NC:(oc + 1) * NC],
                            in_=ot[:],
                        )
    return _split_waits(nc) if split else nc


_NC_CACHE = None


def _get_nc():
    global _NC_CACHE
    if _NC_CACHE is None:
        _NC_CACHE = build_nc()
    return _NC_CACHE


def make_in_maps(query, key, value, key_padding_mask, local_mask,
                 Wq, bq, Wk, bk, Wv, bv, Wo, bo):
    import ml_dtypes
    f = np.float32
    bf = ml_dtypes.bfloat16
    in_maps = []
    for c in range(8):
        b, g = c // 2, c % 2
        gs = slice(g * G, (g + 1) * G)
        mask = (key_padding_mask if g == 0 else local_mask)[b]
        mbias = np.where(mask, NEG, 0.0).astype(f).reshape(8, 128).T  # [128, 8]
        in_maps.append({
            "xqT": np.ascontiguousarray(query[b].T, dtype=bf),
            "xkT": np.ascontiguousarray(key[b].T, dtype=bf),
            "xvT": np.ascontiguousarray(value[b].T, dtype=bf),
            "wqT": np.ascontiguousarray(Wq[gs, :].T, dtype=bf),
            "wkT": np.ascontiguousarray(Wk[gs, :].T, dtype=bf),
            "wvT": np.ascontiguousarray(Wv[gs, :].T, dtype=bf),
            "woT": np.ascontiguousarray(Wo[:, gs].T, dtype=bf),
            "mb": np.ascontiguousarray(mbias),
            "bqc": np.ascontiguousarray(bq[gs].astype(f).reshape(4, 128).T),
            "bkc": np.ascontiguousarray(bk[gs].astype(f).reshape(4, 128).T),
            "bvr": np.ascontiguousarray(bv[gs].astype(bf).reshape(1, G)),
        })
    return in_maps


def kernel(query, key, value, key_padding_mask, local_mask,
           Wq, bq, Wk, bk, Wv, bv, Wo, bo, _trace=False, _tmpdir=None):
    from concourse.bass_utils import run_bass_kernel_spmd

    nc = _get_nc()
    in_maps = make_in_maps(query, key, value, key_padding_mask, local_mask,
                           Wq, bq, Wk, bk, Wv, bv, Wo, bo)
    try:
        res = run_bass_kernel_spmd(nc, in_maps, list(range(8)),
                                   trace=_trace, tmpdir=_tmpdir)
    except Exception:
        # transient device/transport failures have been observed on the
        # axon path; one fresh attempt is cheap relative to a hard fail
        res = run_bass_kernel_spmd(nc, in_maps, list(range(8)),
                                   trace=_trace, tmpdir=_tmpdir)
    outs = [np.asarray(r["out"]) for r in res.results]
    full = np.stack([outs[2 * b] + outs[2 * b + 1] for b in range(B)])
    full += np.asarray(bo, dtype=np.float32)
    if _trace:
        kernel._last_exec_time_ns = res.exec_time_ns
        kernel._last_profile = res.profile_json
    return full.astype(np.float32)



# revision 1
# speedup vs baseline: 1.0060x; 1.0060x over previous
"""Cross multihead attention (global/local masked head groups) on 8 trn2 cores.

Sharding: core c -> (batch b = c//2, head-group g = c%2).
  g=0: heads 0-7  masked by key_padding_mask[b]
  g=1: heads 8-15 masked by local_mask[b]
Each core computes its group's partial output  (attn_out_g @ Wo[:, g*512:(g+1)*512].T)
of shape [T, E]; the host sums the two partials per batch and adds bo.

On-chip layout ("transposed scores" orientation - zero on-chip transposes):
  qT, kT   : [512(j), 1024(t|s)]  feature-major (j = head*64 + d)
  v        : [1024(s), 8*65]      natural, per-head 65-col stripes [v_h | ones]
  scoresT  : [s, t] tiles; mask folded into Exp bias (per-partition = per-s)
  softmax  : no max-subtraction (scores ~ N(0,1)); denominators from the
             ones column of the augmented v matmul; normalization deferred
             to a per-head [64, 512] multiply with a partition-broadcast
             reciprocal row.
  attnT    : [512(j), 1024(t)] -> out = attnT.T @ woT accumulated over j-tiles.
"""

import os
import sys

sys.path.insert(0, "/opt/trn_rl_repo")

import numpy as np

import concourse.bass as bass
import concourse.mybir as mybir
from concourse.tile import TileContext

B, T, S, E, H = 4, 1024, 1024, 1024, 16
DH = E // H            # 64
HH = H // 2            # 8 heads per group
G = HH * DH            # 512 features per group
SCALING = DH ** -0.5
NEG = -30000.0         # exp(x + NEG) == 0.0 in fp32, no LUT edge cases

F32 = mybir.dt.float32
BF = mybir.dt.bfloat16   # tensor-engine operand dtype (1 cyc/row)


def _mm(ap):
    return ap


def _split_waits(nc):
    """TPB ISA structs hold one sem-wait slot. Tile can emit >1 wait per
    instruction (walrus: 'Too many sync wait commands'); hoist all but the
    last wait onto single-wait NOPs on the same engine, inserted just
    before. Timing is unchanged - the waits would have blocked anyway."""
    k = 0
    for f in nc.m.functions:
        for blk in f.blocks:
            new = []
            for inst in blk.instructions:
                si = inst.sync_info
                w = list(si.on_wait) if si else []
                if len(w) > 1:
                    for wait in w[:-1]:
                        nop = mybir.InstNoOp(name=f"nopw-{k}", ins=[], outs=[])
                        k += 1
                        nop.engine = inst.engine
                        nop.sync_info = mybir.SyncInfo(on_wait=[wait], on_update=[])
                        new.append(nop)
                    inst.sync_info = mybir.SyncInfo(
                        on_wait=[w[-1]], on_update=list(si.on_update)
                    )
                new.append(inst)
            blk.instructions = new
    return nc


def build_nc(split=True, phase='all'):
    nc = bass.Bass()

    xqT = nc.dram_tensor("xqT", [E, T], BF, kind="ExternalInput")
    xkT = nc.dram_tensor("xkT", [E, S], BF, kind="ExternalInput")
    xvT = nc.dram_tensor("xvT", [E, S], BF, kind="ExternalInput")
    wqT = nc.dram_tensor("wqT", [E, G], BF, kind="ExternalInput")
    wkT = nc.dram_tensor("wkT", [E, G], BF, kind="ExternalInput")
    wvT = nc.dram_tensor("wvT", [E, G], BF, kind="ExternalInput")
    woT = nc.dram_tensor("woT", [G, E], BF, kind="ExternalInput")
    mb = nc.dram_tensor("mb", [128, 8], F32, kind="ExternalInput")    # [-30000|0] per s
    bqc = nc.dram_tensor("bqc", [128, 4], F32, kind="ExternalInput")  # bq per j-tile col
    bkc = nc.dram_tensor("bkc", [128, 4], F32, kind="ExternalInput")
    bvr = nc.dram_tensor("bvr", [1, G], BF, kind="ExternalInput")    # bv as row
    out = nc.dram_tensor("out", [T, E], F32, kind="ExternalOutput")

    ET, ST, TT = E // 128, S // 128, T // 128   # 8, 8, 8
    JT = G // 128                               # 4 j-tiles
    NC = 512                                    # moving-operand chunk
    TC = T // NC                                # 2 t-chunks

    with TileContext(nc) as tc:
        with (
            tc.tile_pool(name="const", bufs=1) as pc,
            tc.tile_pool(name="persist", bufs=1) as pp,
            tc.tile_pool(name="xin", bufs=ET) as px,
            tc.tile_pool(name="win", bufs=ET) as pw,
            tc.tile_pool(name="exp", bufs=2 * ST) as pe,
            tc.tile_pool(name="outsb", bufs=3) as po,
            tc.tile_pool(name="small", bufs=4) as psm,
            tc.tile_pool(name="psg", bufs=2, space="PSUM") as ppsg,
            tc.tile_pool(name="pssc", bufs=4, space="PSUM") as ppsc,
            tc.tile_pool(name="psav", bufs=2, space="PSUM") as ppsav,
        ):
            # ---- constants ----
            mb_sb = pc.tile([128, 8], F32, name="mb_sb")
            nc.sync.dma_start(out=mb_sb[:], in_=mb[:])
            bq_sb = pc.tile([128, 4], F32, name="bq_sb")
            nc.sync.dma_start(out=bq_sb[:], in_=bqc[:])
            bk_sb = pc.tile([128, 4], F32, name="bk_sb")
            nc.sync.dma_start(out=bk_sb[:], in_=bkc[:])
            bv_sb = pc.tile([1, G], BF, name="bv_sb")
            nc.sync.dma_start(out=bv_sb[:], in_=bvr[:])
            ones_sb = pc.tile([2, 128], BF, name="ones_sb")
            nc.gpsimd.memset(ones_sb[:], 1.0)

            # ---- persistent activations ----
            qT_sb = [pp.tile([128, T], BF, name=f"qT{r}") for r in range(JT)]
            kT_sb = [pp.tile([128, S], BF, name=f"kT{r}") for r in range(JT)]
            v_sb = [pp.tile([128, HH * (DH + 1)], BF, name=f"v{st}") for st in range(ST)]
            aT_sb = [pp.tile([128, T], BF, name=f"aT{r}") for r in range(JT)]
            woT_sb = [pp.tile([128, E], BF, name=f"woT{r}") for r in range(JT)]

            # ---- q/k projections: out[j,t] = sum_e W.T[e,j] X.T[e,t] (+ bias) ----
            for pi, (xdr, wdr, dst, bias) in enumerate((
                (xqT, wqT, qT_sb, bq_sb),
                (xkT, wkT, kT_sb, bk_sb),
            )):
                xt = [px.tile([128, T], BF, tag=f"xe{pi}", name=f"xe{pi}_{et}") for et in range(ET)]
                wt = [pw.tile([128, G], BF, tag=f"we{pi}", name=f"we{pi}_{et}") for et in range(ET)]
                for et in range(ET):
                    nc.sync.dma_start(out=xt[et][:], in_=xdr[et * 128:(et + 1) * 128, :])
                    nc.sync.dma_start(out=wt[et][:], in_=wdr[et * 128:(et + 1) * 128, :])
                for r in range(JT):
                    for c2 in range(TC):
                        ps = ppsg.tile([128, NC], F32, tag="psg", name="ps_proj")
                        for et in range(ET):
                            nc.tensor.matmul(
                                ps[:],
                                lhsT=_mm(wt[et][:, r * 128:(r + 1) * 128]),
                                rhs=_mm(xt[et][:, c2 * NC:(c2 + 1) * NC]),
                                start=(et == 0), stop=(et == ET - 1),
                            )
                        nc.vector.tensor_scalar_add(
                            dst[r][:, c2 * NC:(c2 + 1) * NC], ps[:], bias[:, r:r + 1]
                        )

            # ---- v projection: v[s,d] = sum_e X.T[e,s] Wv.T[e,d] + bv ----
            xt = [px.tile([128, S], BF, tag="xev", name=f"xve{et}") for et in range(ET)]
            wt = [pw.tile([128, G], BF, tag="wev", name=f"wve{et}") for et in range(ET)]
            for et in range(ET):
                nc.sync.dma_start(out=xt[et][:], in_=xvT[et * 128:(et + 1) * 128, :])
                nc.sync.dma_start(out=wt[et][:], in_=wvT[et * 128:(et + 1) * 128, :])
            for st in range(ST):
                ps = ppsg.tile([128, G], F32, tag="psg", name="ps_v")
                for et in range(ET):
                    nc.tensor.matmul(
                        ps[:],
                        lhsT=_mm(xt[et][:, st * 128:(st + 1) * 128]),
                        rhs=_mm(wt[et][:]),
                        start=(et == 0), stop=False,
                    )
                nc.tensor.matmul(  # += ones[1,128].T @ bv[1,512]
                    ps[:], lhsT=_mm(ones_sb[0:1, :]), rhs=_mm(bv_sb[:]),
                    start=False, stop=True,
                )
                # scatter [128, 8, 64] into 65-col stripes; stripe col 64 <- 1.0
                v3 = v_sb[st][:].rearrange("p (h x) -> p h x", x=DH + 1)
                nc.vector.tensor_copy(
                    v3[:, :, 0:DH], ps[:].rearrange("p (h x) -> p h x", x=DH)
                )
                nc.gpsimd.memset(v3[:, :, DH:DH + 1], 1.0)

            for r in range(JT):
                nc.sync.dma_start(out=woT_sb[r][:], in_=woT[r * 128:(r + 1) * 128, :])

            if phase == 'proj':
                for r in range(JT):
                    ot = po.tile([128, T], F32, tag="otp", name=f"otp{r}")
                    nc.vector.tensor_copy(ot[:], qT_sb[r][:])
                    nc.sync.dma_start(out=out[r * 128:(r + 1) * 128, :], in_=ot[:])
                    ot2 = po.tile([128, T], F32, tag="otp", name=f"otp2{r}")
                    nc.vector.tensor_copy(ot2[:], kT_sb[r][:])
                    nc.sync.dma_start(out=out[512 + r * 128:512 + (r + 1) * 128, :], in_=ot2[:])

            # ---- attention ----
            for c in range(TC if phase == 'all' else 0):
                tsl = slice(c * NC, (c + 1) * NC)
                for hp in range(HH // 2):
                    pair = (2 * hp, 2 * hp + 1)
                    expT = {h: [pe.tile([128, NC], BF, tag="exp", name=f"exp_h{h}_s{st}") for st in range(ST)]
                            for h in pair}
                    for st in range(ST):
                        for h in pair:
                            r, po_ = h // 2, (h % 2) * DH
                            ps_s = ppsc.tile([128, NC], F32, tag="sc", name="ps_s")
                            nc.tensor.matmul(
                                ps_s[:],
                                lhsT=_mm(kT_sb[r][po_:po_ + DH, st * 128:(st + 1) * 128]),
                                rhs=_mm(qT_sb[r][po_:po_ + DH, tsl]),
                                start=True, stop=True,
                            )
                            nc.scalar.activation(
                                expT[h][st][:], ps_s[:],
                                mybir.ActivationFunctionType.Exp,
                                bias=mb_sb[:, st:st + 1], scale=SCALING,
                            )
                    for h in pair:
                        r, po_ = h // 2, (h % 2) * DH
                        ps_o = ppsav.tile([DH + 1, NC], F32, tag="av", name="ps_o")
                        for st in range(ST):
                            nc.tensor.matmul(
                                ps_o[:],
                                lhsT=_mm(v_sb[st][:, h * (DH + 1):(h + 1) * (DH + 1)]),
                                rhs=_mm(expT[h][st][:]),
                                start=(st == 0), stop=(st == ST - 1),
                            )
                        rec = psm.tile([1, NC], F32, tag="rec", name="rec")
                        nc.vector.reciprocal(rec[:], ps_o[DH:DH + 1, :])
                        # broadcast rec across 64 partitions at ~fp32 precision:
                        # hi = bf16(rec), lo = bf16(rec - hi);  ones[2,64].T @ [hi;lo]
                        # sums hi+lo in fp32 PSUM.
                        rhi = psm.tile([1, NC], BF, tag="rhi", name="rhi")
                        nc.vector.tensor_copy(rhi[:], rec[:])
                        rlo = psm.tile([1, NC], BF, tag="rlo", name="rlo")
                        nc.vector.tensor_sub(rlo[:], rec[:], rhi[:])
                        ps_b = ppsc.tile([DH, NC], F32, tag="sc", name="ps_b")
                        nc.tensor.matmul(ps_b[:], lhsT=ones_sb[0:1, 0:DH],
                                         rhs=rhi[:], start=True, stop=False)
                        nc.tensor.matmul(ps_b[:], lhsT=ones_sb[0:1, 0:DH],
                                         rhs=rlo[:], start=False, stop=True)
                        rb = psm.tile([DH, NC], F32, tag="rb", name="rb")
                        nc.vector.tensor_copy(rb[:], ps_b[:])
                        nc.vector.tensor_mul(
                            aT_sb[r][po_:po_ + DH, tsl],
                            ps_o[0:DH, :],
                            rb[:],
                        )
                # ---- output projection for this chunk's t-tiles ----
                for tt in range(c * 4, c * 4 + 4):
                    for oc in range(2):
                        ps_u = ppsg.tile([128, NC], F32, tag="psg", name="ps_u")
                        for r in range(JT):
                            nc.tensor.matmul(
                                ps_u[:],
                                lhsT=_mm(aT_sb[r][:, tt * 128:(tt + 1) * 128]),
                                rhs=_mm(woT_sb[r][:, oc * NC:(oc + 1) * NC]),
                                start=(r == 0), stop=(r == JT - 1),
                            )
                        ot = po.tile([128, NC], F32, tag="ot", name="ot")
                        nc.vector.tensor_copy(ot[:], ps_u[:])
                        nc.sync.dma_start(
                            out=out[tt * 128:(tt + 1) * 128, oc * NC:(oc + 1) * NC],
                            in_=ot[:],
                        )
    return _split_waits(nc) if split else nc


_NC_CACHE = None


def _get_nc():
    global _NC_CACHE
    if _NC_CACHE is None:
        _NC_CACHE = build_nc()
    return _NC_CACHE


def make_in_maps(query, key, value, key_padding_mask, local_mask,
                 Wq, bq, Wk, bk, Wv, bv, Wo, bo):
    import ml_dtypes
    f = np.float32
    bf = ml_dtypes.bfloat16
    in_maps = []
    for c in range(8):
        b, g = c // 2, c % 2
        gs = slice(g * G, (g + 1) * G)
        mask = (key_padding_mask if g == 0 else local_mask)[b]
        mbias = np.where(mask, NEG, 0.0).astype(f).reshape(8, 128).T  # [128, 8]
        in_maps.append({
            "xqT": np.ascontiguousarray(query[b].T, dtype=bf),
            "xkT": np.ascontiguousarray(key[b].T, dtype=bf),
            "xvT": np.ascontiguousarray(value[b].T, dtype=bf),
            "wqT": np.ascontiguousarray(Wq[gs, :].T, dtype=bf),
            "wkT": np.ascontiguousarray(Wk[gs, :].T, dtype=bf),
            "wvT": np.ascontiguousarray(Wv[gs, :].T, dtype=bf),
            "woT": np.ascontiguousarray(Wo[:, gs].T, dtype=bf),
            "mb": np.ascontiguousarray(mbias),
            "bqc": np.ascontiguousarray(bq[gs].astype(f).reshape(4, 128).T),
            "bkc": np.ascontiguousarray(bk[gs].astype(f).reshape(4, 128).T),
            "bvr": np.ascontiguousarray(bv[gs].astype(bf).reshape(1, G)),
        })
    return in_maps


def kernel(query, key, value, key_padding_mask, local_mask,
           Wq, bq, Wk, bk, Wv, bv, Wo, bo, _trace=False, _tmpdir=None):
    from concourse.bass_utils import run_bass_kernel_spmd

    nc = _get_nc()
    in_maps = make_in_maps(query, key, value, key_padding_mask, local_mask,
                           Wq, bq, Wk, bk, Wv, bv, Wo, bo)
    try:
        res = run_bass_kernel_spmd(nc, in_maps, list(range(8)),
                                   trace=_trace, tmpdir=_tmpdir)
    except Exception:
        # transient device/transport failures have been observed on the
        # axon path; one fresh attempt is cheap relative to a hard fail
        res = run_bass_kernel_spmd(nc, in_maps, list(range(8)),
                                   trace=_trace, tmpdir=_tmpdir)
    outs = [np.asarray(r["out"]) for r in res.results]
    full = np.stack([outs[2 * b] + outs[2 * b + 1] for b in range(B)])
    full += np.asarray(bo, dtype=np.float32)
    if _trace:
        kernel._last_exec_time_ns = res.exec_time_ns
        kernel._last_profile = res.profile_json
    return full.astype(np.float32)

